# revision 10
# baseline (speedup 1.0000x reference)
"""GNN message passing (GraphConvolution) on 8 TRN2 NeuronCores.

reference:
    support = x @ W                                   # [N, H]
    msgs    = support[edge_src] * edge_w[:, None]     # [E, H]
    agg     = segment_sum(msgs, edge_dst, N)          # [N, H]
    out     = relu(agg + b)

v3 strategy (dst-node 1D sharding; src-sharded support + AllGather):
  - Core c owns dst nodes [c*NPC, (c+1)*NPC).
  - Phase 1 is src-sharded: core c computes support rows for ITS x shard
    (12500 rows, padded 12800) in 4 quarter-blocks; after each block an
    AllGather builds table_k (25600 rows = 8 ranks x 3200), so phase-2
    gathers for run k start as soon as AG_k lands.
  - Host permutes each core's dst nodes into subtiles of <=64 nodes such
    that every (subtile, run) bucket has <=256 edges -> EXACTLY 2 chunks
    of 128 gather slots per bucket on every core (uniform SPMD shape,
    ~16% fewer gathered rows than the maxed-over-cores v2 layout).
  - Phase 2 is Q7-descriptor-generation bound (~2.4ns/row at 4-way SWDGE
    queue concurrency, measured): all per-call inputs (idx plane, meta
    plane) are preloaded ONCE, and num_idxs registers are hoisted, so the
    Pool engine issues gathers back-to-back with no interleaved waits
    (v2 lost 2x to issue-side serialization).
  - Weighted one-hot indicator S[e, d] = w_e * (dstloc_e == d), SUB=64
    wide, built on VectorE (2x_1P mode, never contends with GpSimd);
    each 128-slot chunk reduced with one TensorE matmul into a [128,512]
    PSUM supertile (8 subtiles); supers processed in groups of 6 ordered
    (run, super) to match the AG cadence with <=8 PSUM banks.
  - Epilogue: ScalarE relu(psum + b) -> outT [H, n_sub*64] -> host
    inverse-permutation.
"""

import os

import ml_dtypes
import numpy as np

import concourse.bass as bass
import concourse.mybir as mybir
import concourse.tile as tile
from concourse import bacc
from concourse.bass_utils import run_bass_kernel_spmd
from concourse.library_config import mlp as _mlp_lib

BF16 = ml_dtypes.bfloat16

N_CORES = 8
NPC = 12500  # dst nodes per core
SUB = 64  # dst nodes per subtile (one-hot width)
CAP = 256  # max edges per (subtile, run) bucket = 2 chunks
NODE_CAP = 64  # nodes per subtile
PSUM_COLS = 512  # one PSUM bank: 8 subtiles per supertile
SPS = PSUM_COLS // SUB  # subtiles per supertile (8)
GROUP = 6  # supertiles per scheduling group (psum bufs)

SHARD_PAD = 12800  # src rows per core, padded
N_RUNS = 4
TBLK = SHARD_PAD // N_RUNS  # 3200 shard rows per AG quarter
TROWS = TBLK * N_CORES  # 25600 rows per all-gathered table
XBLK = TBLK  # phase-1 row block == AG quarter


def _ceil_div(a, b):
    return (a + b - 1) // b


def _pack_core(dvec, n_sub):
    """Greedy 4-run balanced packing of this core's dst nodes into n_sub
    subtiles: per-bucket load <= CAP, per-subtile nodes <= NODE_CAP.
    dvec: [NPC, N_RUNS] per-node degree by run. Returns sub_of [NPC] or None."""
    tot = dvec.sum(axis=1)
    order = np.argsort(-tot, kind="stable")
    loads = np.zeros((n_sub, N_RUNS), np.int64)
    counts = np.zeros(n_sub, np.int64)
    sub_of = np.full(NPC, -1, np.int64)
    for n in order:
        d = dvec[n]
        cand = (counts < NODE_CAP) & (loads + d <= CAP).all(axis=1)
        if not cand.any():
            return None
        score = (loads + d).max(axis=1).astype(np.float64) + counts * 1e-4
        score[~cand] = np.inf
        b = int(np.argmin(score))
        loads[b] += d
        counts[b] += 1
        sub_of[n] = b
    return sub_of


def prepare(x, edge_src, edge_dst, edge_w, W, b):
    """Host-side sharding/routing. Returns (cfg, in_maps, col_of)."""
    n_nodes, nfeat = x.shape
    nhid = W.shape[1]
    assert n_nodes == N_CORES * NPC

    src = np.asarray(edge_src).astype(np.int64)
    dst = np.asarray(edge_dst).astype(np.int64)
    ew = np.asarray(edge_w).astype(np.float32)

    # src -> (run, row): run 0 = tableA (quarter 0, rank-major); runs 1-3 =
    # consecutive 25600-row slices of tableB (quarters 1-3, rank-major).
    s_rank = src // NPC
    s_loc = src % NPC
    s_q = s_loc // TBLK
    s_w = s_loc % TBLK
    rb = s_rank * (3 * TBLK) + (s_q - 1) * TBLK + s_w  # row in tableB (q>=1)
    s_run = np.where(s_q == 0, 0, 1 + rb // TROWS)
    s_row = np.where(s_q == 0, s_rank * TBLK + s_w, rb % TROWS)  # int16-safe

    core_of = dst // NPC

    # --- per-core balanced dst permutation ---
    n_sub = int(os.environ.get("GNN_NSUB", "200"))
    dvecs = []
    for c in range(N_CORES):
        m = core_of == c
        d_loc = dst[m] - c * NPC
        dvec = np.zeros((NPC, N_RUNS), np.int64)
        np.add.at(dvec, (d_loc, s_run[m]), 1)
        dvecs.append((m, d_loc, dvec))
    while True:
        percore = []
        for c in range(N_CORES):
            m, d_loc, dvec = dvecs[c]
            sub_of = _pack_core(dvec, n_sub)
            if sub_of is None:
                break
            percore.append((m, d_loc, sub_of))
        if len(percore) == N_CORES:
            break
        n_sub += 4

    n_super = _ceil_div(n_sub, SPS)
    # call program order: groups of supertiles, (run, super) within group
    call_order = []  # list of (S, t)
    for g0 in range(0, n_super, GROUP):
        gs = list(range(g0, min(g0 + GROUP, n_super)))
        for t in range(N_RUNS):
            for S in gs:
                call_order.append((S, t))
    # chunks per call: subtiles in super * 2 (except last, partial super)
    calls = []
    chunk_off = 0
    for S, t in call_order:
        subs = list(range(S * SPS, min(S * SPS + SPS, n_sub)))
        gc = 2 * len(subs)
        sub_local = []
        for s in subs:
            sub_local.extend([s - S * SPS, s - S * SPS])
        calls.append(
            dict(S=S, t=t, n_chunks=gc, chunk_off=chunk_off, sub_local=sub_local)
        )
        chunk_off += gc
    nchunk = chunk_off
    e_pad = nchunk * 128
    assert nchunk == n_sub * N_RUNS * 2

    # chunk slot ranges per (subtile, run): chunk index within program order
    chunk_of = {}
    pos = 0
    for call in calls:
        S, t = call["S"], call["t"]
        subs = range(S * SPS, min(S * SPS + SPS, n_sub))
        for s in subs:
            chunk_of[(s, t)] = pos
            pos += 2
    assert pos == nchunk

    in_maps = []
    w_bf = np.ascontiguousarray(np.asarray(W, np.float32).astype(BF16))
    bias = np.asarray(b, np.float32).reshape(nhid, 1).copy()
    iota = np.tile(np.arange(SUB, dtype=np.float32).astype(BF16)[None, :], (128, 1))
    x_np = np.asarray(x, np.float32)
    col_of = np.zeros((N_CORES, NPC), np.int64)
    for c in range(N_CORES):
        m, d_loc, sub_of = percore[c]
        # within-subtile slot assignment (stable by node id)
        slot = np.zeros(NPC, np.int64)
        for s in range(n_sub):
            nodes = np.nonzero(sub_of == s)[0]
            slot[nodes] = np.arange(len(nodes))
        col_of[c] = sub_of * SUB + slot

        e_sub = sub_of[d_loc]  # subtile of each edge
        e_dl = slot[d_loc]  # within-subtile one-hot position
        e_t = s_run[m]
        e_row = s_row[m]
        e_w = ew[m]
        key = e_sub * N_RUNS + e_t
        order = np.argsort(key, kind="stable")
        e_sub, e_dl, e_t, e_row, e_w = (
            a[order] for a in (e_sub, e_dl, e_t, e_row, e_w)
        )
        cnt = np.bincount(key[order], minlength=n_sub * N_RUNS)
        seg_start = np.zeros(n_sub * N_RUNS + 1, np.int64)
        np.cumsum(cnt, out=seg_start[1:])

        idx_pad = np.zeros(e_pad, np.int16)
        dl_pad = np.zeros(e_pad, np.float32)
        ww_pad = np.zeros(e_pad, np.float32)
        for s in range(n_sub):
            for t in range(N_RUNS):
                k = s * N_RUNS + t
                a0, a1 = seg_start[k], seg_start[k + 1]
                n = a1 - a0
                if n == 0:
                    continue
                off = chunk_of[(s, t)] * 128
                idx_pad[off : off + n] = e_row[a0:a1].astype(np.int16)
                dl_pad[off : off + n] = e_dl[a0:a1].astype(np.float32)
                ww_pad[off : off + n] = e_w[a0:a1]

        xT_shard = np.zeros((nfeat, SHARD_PAD), BF16)
        xT_shard[:, :NPC] = x_np[c * NPC : (c + 1) * NPC, :].T.astype(BF16)
        in_maps.append(
            dict(
                xT=xT_shard,
                wmat=w_bf,
                bias=bias,
                iota=iota,
                idx=np.ascontiguousarray(np.tile(idx_pad.reshape(-1, 16).T, (8, 1))),
                # dl/ew duplicated in adjacent pairs (DVE 2x_1P needs step-1
                # 2-elem innermost runs), merged into one plane
                meta=np.ascontiguousarray(
                    np.concatenate(
                        [
                            np.repeat(
                                a.reshape(-1, 128).T.astype(BF16)[:, :, None], 2, 2
                            )
                            for a in (dl_pad, ww_pad)
                        ],
                        axis=2,
                    )
                ),
            )
        )

    cfg = dict(
        nfeat=nfeat,
        nhid=nhid,
        n_sub=n_sub,
        n_super=n_super,
        nchunk=nchunk,
        e_pad=e_pad,
        calls=calls,
    )
    return cfg, in_maps, col_of


def build_bass(cfg):
    F, H = cfg["nfeat"], cfg["nhid"]
    KC = F // 128
    n_sub = cfg["n_sub"]
    OUTC = n_sub * SUB
    assert F % 128 == 0 and H == 128
    n_queues = int(os.environ.get("GNN_QUEUES", "4"))

    nc = bacc.Bacc(
        "TRN2",
        target_bir_lowering=False,
        debug=False,
        enable_asserts=True,
        num_devices=N_CORES,
        num_swdge_queues=n_queues,
    )
    f32, bf16, i16 = mybir.dt.float32, mybir.dt.bfloat16, mybir.dt.int16
    xT = nc.dram_tensor("xT", [F, SHARD_PAD], bf16, kind="ExternalInput")
    wmat = nc.dram_tensor("wmat", [F, H], bf16, kind="ExternalInput")
    bias = nc.dram_tensor("bias", [H, 1], f32, kind="ExternalInput")
    iota = nc.dram_tensor("iota", [128, SUB], bf16, kind="ExternalInput")
    idx = nc.dram_tensor("idx", [128, cfg["e_pad"] // 16], i16, kind="ExternalInput")
    meta = nc.dram_tensor("meta", [128, cfg["nchunk"], 4], bf16, kind="ExternalInput")
    outT = nc.dram_tensor("outT", [H, OUTC], f32, kind="ExternalOutput")

    AF = mybir.ActivationFunctionType
    with tile.TileContext(nc) as tc:
        with (
            tc.tile_pool(name="const", bufs=1) as cpool,
            tc.tile_pool(name="xt", bufs=4) as xpool,
            tc.tile_pool(name="sup", bufs=2) as spool,
            tc.tile_pool(name="gath", bufs=10) as gpool,
            tc.tile_pool(name="ind", bufs=6) as ipool,
            tc.tile_pool(name="outb", bufs=2) as opool,
            tc.tile_pool(name="psum", bufs=8, space="PSUM") as ppool,
            tc.tile_pool(name="dram", bufs=1, space="DRAM") as dpool,
        ):
            nc.gpsimd.load_library(_mlp_lib)
            # Warm up the collective stream immediately: the first collective
            # pays a ~36us barrier/bootstrap rendezvous; issuing a tiny dummy
            # AllGather first overlaps that cost with phase 1.
            warm_in = dpool.tile([16, 16], bf16, name="warm_in")
            warm_out = dpool.tile(
                [16 * N_CORES, 16], bf16, addr_space="Shared", name="warm_out"
            )
            nc.gpsimd.collective_compute(
                "AllGather",
                mybir.AluOpType.bypass,
                replica_groups=[list(range(N_CORES))],
                ins=[warm_in.opt()],
                outs=[warm_out.opt()],
            )
            # 2 asymmetric AllGathers: quarter 0 -> small early table (run 0);
            # quarters 1-3 -> one big table holding runs 1-3. ~28us fixed cost
            # per collective makes more, finer AGs a net loss.
            shard0 = dpool.tile([TBLK, H], bf16, name="shard0")
            shard123 = dpool.tile([3 * TBLK, H], bf16, name="shard123")
            tableA = dpool.tile([TROWS, H], bf16, addr_space="Shared", name="tableA")
            tableB = dpool.tile(
                [3 * TROWS, H], bf16, addr_space="Shared", name="tableB"
            )

            w_sb = cpool.tile([128, KC, H], bf16)
            nc.sync.dma_start(
                out=w_sb[:], in_=wmat.ap().rearrange("(c k) h -> k c h", k=128)
            )
            bias_sb = cpool.tile([H, 1], f32)
            nc.sync.dma_start(out=bias_sb[:], in_=bias.ap())
            iota_sb = cpool.tile([128, SUB], bf16)
            nc.scalar.dma_start(out=iota_sb[:], in_=iota.ap())
            # preload the full idx / meta planes once (scalar HWDGE queue so
            # the sync engine starts the phase-1 x loads immediately)
            idx_sb = cpool.tile([128, cfg["e_pad"] // 16], i16)
            nc.scalar.dma_start(out=idx_sb[:], in_=idx.ap())
            meta_sb = cpool.tile([128, cfg["nchunk"], 4], bf16)
            nc.scalar.dma_start(out=meta_sb[:], in_=meta.ap())

            # ---- phase 1: support_shard = x_shard @ W; AG0 then AG1 ----
            for blk in range(N_RUNS):
                xts = []
                for kc in range(KC):
                    xt = xpool.tile([128, XBLK], bf16, tag=f"xt{kc}")
                    nc.sync.dma_start(
                        out=xt[:],
                        in_=xT.ap()[
                            kc * 128 : (kc + 1) * 128, blk * XBLK : (blk + 1) * XBLK
                        ],
                    )
                    xts.append(xt)
                st = spool.tile([128, XBLK], bf16)
                for i in range(XBLK // 128):
                    col = i * 128
                    ps = ppool.tile([128, 128], f32, tag="agg_ps")
                    for kc in range(KC):
                        nc.tensor.matmul(
                            ps[:],
                            xts[kc][:, col : col + 128],
                            w_sb[:, kc, :],
                            start=(kc == 0),
                            stop=(kc == KC - 1),
                        )
                    nc.scalar.activation(
                        out=st[:, col : col + 128], in_=ps[:], func=AF.Copy
                    )
                shard_dst = (
                    shard0[:]
                    if blk == 0
                    else shard123[(blk - 1) * TBLK : blk * TBLK, :]
                )
                nc.sync.dma_start(
                    out=shard_dst.rearrange("(i p) h -> p i h", p=128),
                    in_=st[:].rearrange("p (i h) -> p i h", h=H),
                )
                if blk == 0:
                    nc.gpsimd.collective_compute(
                        "AllGather",
                        mybir.AluOpType.bypass,
                        replica_groups=[list(range(N_CORES))],
                        ins=[shard0.opt()],
                        outs=[tableA.opt()],
                    )
                elif blk == N_RUNS - 1:
                    nc.gpsimd.collective_compute(
                        "AllGather",
                        mybir.AluOpType.bypass,
                        replica_groups=[list(range(N_CORES))],
                        ins=[shard123.opt()],
                        outs=[tableB.opt()],
                    )

            # ---- phase 2: gather + weighted-one-hot matmul segment sum ----
            # one psum tile per supertile; calls arrive in (group, t, S) order
            psums = {}
            mm_count = {}
            reg_cache = {}
            for ci, call in enumerate(cfg["calls"]):
                S, t, Gc = call["S"], call["t"], call["n_chunks"]
                L = Gc * 128
                q = ci % n_queues
                if S not in psums:
                    psums[S] = ppool.tile(
                        [128, PSUM_COLS], f32, name=f"agg_ps{S}", tag="agg_ps"
                    )
                    mm_count[S] = 0
                pss = psums[S]
                if L not in reg_cache:
                    reg_cache[L] = nc.gpsimd.to_reg(L)
                gt = gpool.tile([128, Gc, H], bf16)
                c0 = call["chunk_off"] * 8  # idx col = chunk_off*128/16
                src_ap = (
                    tableA[:]
                    if t == 0
                    else tableB[(t - 1) * TROWS : t * TROWS, :]
                )
                nc.gpsimd.dma_gather(
                    gt[:],
                    src_ap,
                    idx_sb[:, c0 : c0 + L // 16],
                    L,
                    reg_cache[L],
                    H,
                    single_packet=False,
                    queue_num=q,
                )
                mt = meta_sb[:, call["chunk_off"] : call["chunk_off"] + Gc, :]
                ind = ipool.tile([128, Gc, SUB], bf16)
                ind4 = ind[:].rearrange("p g (a b) -> p g a b", b=2)
                nc.vector.tensor_tensor(
                    out=ind4,
                    in0=iota_sb[:]
                    .rearrange("p (a b) -> p a b", b=2)[:, None, :, :]
                    .to_broadcast([128, Gc, SUB // 2, 2]),
                    in1=mt[:, :, None, 0:2].to_broadcast([128, Gc, SUB // 2, 2]),
                    op=mybir.AluOpType.is_equal,
                )
                nc.vector.tensor_tensor(
                    out=ind4,
                    in0=ind4,
                    in1=mt[:, :, None, 2:4].to_broadcast([128, Gc, SUB // 2, 2]),
                    op=mybir.AluOpType.mult,
                )
                total_S = 2 * N_RUNS * len(set(call["sub_local"]))
                for j in range(Gc):
                    so = call["sub_local"][j]
                    nc.tensor.matmul(
                        pss[:, so * SUB : (so + 1) * SUB],
                        gt[:, j, :],
                        ind[:, j, :],
                        start=(mm_count[S] == 0),
                        stop=(mm_count[S] == total_S - 1),
                    )
                    mm_count[S] += 1
                if mm_count[S] == total_S:
                    # supertile complete: epilogue
                    wS = min(PSUM_COLS, OUTC - S * PSUM_COLS)
                    ob = opool.tile([H, PSUM_COLS], f32)
                    nc.scalar.activation(
                        out=ob[:, :wS],
                        in_=pss[:, :wS],
                        func=AF.Relu,
                        bias=bias_sb[:],
                        scale=1.0,
                    )
                    nc.sync.dma_start(
                        out=outT.ap()[:, S * PSUM_COLS : S * PSUM_COLS + wS],
                        in_=ob[:, :wS],
                    )
                    del psums[S]
    nc.compile()
    return nc


def kernel(x, edge_src, edge_dst, edge_w, W, b):
    x = np.asarray(x)
    cfg, in_maps, col_of = prepare(x, edge_src, edge_dst, edge_w, W, b)
    nc = build_bass(cfg)
    want_trace = bool(int(os.environ.get("GNN_TRACE", "0")))
    core_ids = list(range(N_CORES))
    if want_trace:
        try:
            res = run_bass_kernel_spmd(nc, in_maps, core_ids=core_ids, trace=True)
        except Exception as e:
            print(f"traced run failed ({e}); retrying without trace")
            res = run_bass_kernel_spmd(nc, in_maps, core_ids=core_ids, trace=False)
    else:
        res = run_bass_kernel_spmd(nc, in_maps, core_ids=core_ids, trace=False)
    kernel.last_result = res
    out = np.empty((N_CORES * NPC, 128), np.float32)
    for c, r in enumerate(res.results):
        out[c * NPC : (c + 1) * NPC, :] = r["outT"].T[col_of[c], :]
    return np.ascontiguousarray(out)


kernel.last_result = None


# revision 11
# speedup vs baseline: 1.0261x; 1.0261x over previous
"""GNN message passing (GraphConvolution) on 8 TRN2 NeuronCores.

reference:
    support = x @ W                                   # [N, H]
    msgs    = support[edge_src] * edge_w[:, None]     # [E, H]
    agg     = segment_sum(msgs, edge_dst, N)          # [N, H]
    out     = relu(agg + b)

v3 strategy (dst-node 1D sharding; src-sharded support + AllGather):
  - Core c owns dst nodes [c*NPC, (c+1)*NPC).
  - Phase 1 is src-sharded: core c computes support rows for ITS x shard
    (12500 rows, padded 12800) in 4 quarter-blocks; after each block an
    AllGather builds table_k (25600 rows = 8 ranks x 3200), so phase-2
    gathers for run k start as soon as AG_k lands.
  - Host permutes each core's dst nodes into subtiles of <=64 nodes such
    that every (subtile, run) bucket has <=256 edges -> EXACTLY 2 chunks
    of 128 gather slots per bucket on every core (uniform SPMD shape,
    ~16% fewer gathered rows than the maxed-over-cores v2 layout).
  - Phase 2 is Q7-descriptor-generation bound (~2.4ns/row at 4-way SWDGE
    queue concurrency, measured): all per-call inputs (idx plane, meta
    plane) are preloaded ONCE, and num_idxs registers are hoisted, so the
    Pool engine issues gathers back-to-back with no interleaved waits
    (v2 lost 2x to issue-side serialization).
  - Weighted one-hot indicator S[e, d] = w_e * (dstloc_e == d), SUB=64
    wide, built on VectorE (2x_1P mode, never contends with GpSimd);
    each 128-slot chunk reduced with one TensorE matmul into a [128,512]
    PSUM supertile (8 subtiles); supers processed in groups of 6 ordered
    (run, super) to match the AG cadence with <=8 PSUM banks.
  - Epilogue: ScalarE relu(psum + b) -> outT [H, n_sub*64] -> host
    inverse-permutation.
"""

import os

import ml_dtypes
import numpy as np

import concourse.bass as bass
import concourse.mybir as mybir
import concourse.tile as tile
from concourse import bacc
from concourse.bass_utils import run_bass_kernel_spmd
from concourse.library_config import mlp as _mlp_lib

BF16 = ml_dtypes.bfloat16

N_CORES = 8
NPC = 12500  # dst nodes per core
SUB = 64  # dst nodes per subtile (one-hot width)
CAP = 256  # max edges per (subtile, run) bucket = 2 chunks
NODE_CAP = 64  # nodes per subtile
PSUM_COLS = 512  # one PSUM bank: 8 subtiles per supertile
SPS = PSUM_COLS // SUB  # subtiles per supertile (8)
GROUP = 6  # supertiles per scheduling group (psum bufs)

SHARD_PAD = 12800  # src rows per core, padded
N_RUNS = 4
TBLK = SHARD_PAD // N_RUNS  # 3200 shard rows per AG quarter
TROWS = TBLK * N_CORES  # 25600 rows per all-gathered table
XBLK = TBLK  # phase-1 row block == AG quarter


def _ceil_div(a, b):
    return (a + b - 1) // b


def _pack_core(dvec, n_sub):
    """Greedy 4-run balanced packing of this core's dst nodes into n_sub
    subtiles: per-bucket load <= CAP, per-subtile nodes <= NODE_CAP.
    dvec: [NPC, N_RUNS] per-node degree by run. Returns sub_of [NPC] or None."""
    tot = dvec.sum(axis=1)
    order = np.argsort(-tot, kind="stable")
    loads = np.zeros((n_sub, N_RUNS), np.int64)
    counts = np.zeros(n_sub, np.int64)
    sub_of = np.full(NPC, -1, np.int64)
    for n in order:
        d = dvec[n]
        cand = (counts < NODE_CAP) & (loads + d <= CAP).all(axis=1)
        if not cand.any():
            return None
        score = (loads + d).max(axis=1).astype(np.float64) + counts * 1e-4
        score[~cand] = np.inf
        b = int(np.argmin(score))
        loads[b] += d
        counts[b] += 1
        sub_of[n] = b
    return sub_of


def prepare(x, edge_src, edge_dst, edge_w, W, b):
    """Host-side sharding/routing. Returns (cfg, in_maps, col_of)."""
    n_nodes, nfeat = x.shape
    nhid = W.shape[1]
    assert n_nodes == N_CORES * NPC

    src = np.asarray(edge_src).astype(np.int64)
    dst = np.asarray(edge_dst).astype(np.int64)
    ew = np.asarray(edge_w).astype(np.float32)

    # src -> (run, row): run 0 = tableA (quarter 0, rank-major); runs 1-3 =
    # consecutive 25600-row slices of tableB (quarters 1-3, rank-major).
    s_rank = src // NPC
    s_loc = src % NPC
    s_q = s_loc // TBLK
    s_w = s_loc % TBLK
    rb = s_rank * (3 * TBLK) + (s_q - 1) * TBLK + s_w  # row in tableB (q>=1)
    s_run = np.where(s_q == 0, 0, 1 + rb // TROWS)
    s_row = np.where(s_q == 0, s_rank * TBLK + s_w, rb % TROWS)  # int16-safe

    core_of = dst // NPC

    # --- per-core balanced dst permutation ---
    n_sub = int(os.environ.get("GNN_NSUB", "200"))
    dvecs = []
    for c in range(N_CORES):
        m = core_of == c
        d_loc = dst[m] - c * NPC
        dvec = np.zeros((NPC, N_RUNS), np.int64)
        np.add.at(dvec, (d_loc, s_run[m]), 1)
        dvecs.append((m, d_loc, dvec))
    while True:
        percore = []
        for c in range(N_CORES):
            m, d_loc, dvec = dvecs[c]
            sub_of = _pack_core(dvec, n_sub)
            if sub_of is None:
                break
            percore.append((m, d_loc, sub_of))
        if len(percore) == N_CORES:
            break
        n_sub += 4

    n_super = _ceil_div(n_sub, SPS)
    # call program order: groups of supertiles, (run, super) within group
    call_order = []  # list of (S, t)
    for g0 in range(0, n_super, GROUP):
        gs = list(range(g0, min(g0 + GROUP, n_super)))
        for t in range(N_RUNS):
            for S in gs:
                call_order.append((S, t))
    # chunks per call: subtiles in super * 2 (except last, partial super)
    calls = []
    chunk_off = 0
    for S, t in call_order:
        subs = list(range(S * SPS, min(S * SPS + SPS, n_sub)))
        gc = 2 * len(subs)
        sub_local = []
        for s in subs:
            sub_local.extend([s - S * SPS, s - S * SPS])
        calls.append(
            dict(S=S, t=t, n_chunks=gc, chunk_off=chunk_off, sub_local=sub_local)
        )
        chunk_off += gc
    nchunk = chunk_off
    e_pad = nchunk * 128
    assert nchunk == n_sub * N_RUNS * 2

    # chunk slot ranges per (subtile, run): chunk index within program order
    chunk_of = {}
    pos = 0
    for call in calls:
        S, t = call["S"], call["t"]
        subs = range(S * SPS, min(S * SPS + SPS, n_sub))
        for s in subs:
            chunk_of[(s, t)] = pos
            pos += 2
    assert pos == nchunk

    in_maps = []
    w_bf = np.ascontiguousarray(np.asarray(W, np.float32).astype(BF16))
    bias = np.asarray(b, np.float32).reshape(nhid, 1).copy()
    iota = np.tile(np.arange(SUB, dtype=np.float32).astype(BF16)[None, :], (128, 1))
    x_np = np.asarray(x, np.float32)
    col_of = np.zeros((N_CORES, NPC), np.int64)
    for c in range(N_CORES):
        m, d_loc, sub_of = percore[c]
        # within-subtile slot assignment (stable by node id)
        slot = np.zeros(NPC, np.int64)
        for s in range(n_sub):
            nodes = np.nonzero(sub_of == s)[0]
            slot[nodes] = np.arange(len(nodes))
        col_of[c] = sub_of * SUB + slot

        e_sub = sub_of[d_loc]  # subtile of each edge
        e_dl = slot[d_loc]  # within-subtile one-hot position
        e_t = s_run[m]
        e_row = s_row[m]
        e_w = ew[m]
        key = e_sub * N_RUNS + e_t
        order = np.argsort(key, kind="stable")
        e_sub, e_dl, e_t, e_row, e_w = (
            a[order] for a in (e_sub, e_dl, e_t, e_row, e_w)
        )
        cnt = np.bincount(key[order], minlength=n_sub * N_RUNS)
        seg_start = np.zeros(n_sub * N_RUNS + 1, np.int64)
        np.cumsum(cnt, out=seg_start[1:])

        idx_pad = np.zeros(e_pad, np.int16)
        dl_pad = np.zeros(e_pad, np.float32)
        ww_pad = np.zeros(e_pad, np.float32)
        for s in range(n_sub):
            for t in range(N_RUNS):
                k = s * N_RUNS + t
                a0, a1 = seg_start[k], seg_start[k + 1]
                n = a1 - a0
                if n == 0:
                    continue
                off = chunk_of[(s, t)] * 128
                idx_pad[off : off + n] = e_row[a0:a1].astype(np.int16)
                dl_pad[off : off + n] = e_dl[a0:a1].astype(np.float32)
                ww_pad[off : off + n] = e_w[a0:a1]

        xT_shard = np.zeros((nfeat, SHARD_PAD), BF16)
        xT_shard[:, :NPC] = x_np[c * NPC : (c + 1) * NPC, :].T.astype(BF16)
        in_maps.append(
            dict(
                xT=xT_shard,
                wmat=w_bf,
                bias=bias,
                iota=iota,
                idx=np.ascontiguousarray(np.tile(idx_pad.reshape(-1, 16).T, (8, 1))),
                # dl/ew duplicated in adjacent pairs (DVE 2x_1P needs step-1
                # 2-elem innermost runs), merged into one plane
                meta=np.ascontiguousarray(
                    np.concatenate(
                        [
                            np.repeat(
                                a.reshape(-1, 128).T.astype(BF16)[:, :, None], 2, 2
                            )
                            for a in (dl_pad, ww_pad)
                        ],
                        axis=2,
                    )
                ),
            )
        )

    cfg = dict(
        nfeat=nfeat,
        nhid=nhid,
        n_sub=n_sub,
        n_super=n_super,
        nchunk=nchunk,
        e_pad=e_pad,
        calls=calls,
    )
    return cfg, in_maps, col_of


def build_bass(cfg):
    F, H = cfg["nfeat"], cfg["nhid"]
    KC = F // 128
    n_sub = cfg["n_sub"]
    OUTC = n_sub * SUB
    assert F % 128 == 0 and H == 128
    n_queues = int(os.environ.get("GNN_QUEUES", "4"))

    nc = bacc.Bacc(
        "TRN2",
        target_bir_lowering=False,
        debug=False,
        enable_asserts=True,
        num_devices=N_CORES,
        num_swdge_queues=n_queues,
    )
    f32, bf16, i16 = mybir.dt.float32, mybir.dt.bfloat16, mybir.dt.int16
    xT = nc.dram_tensor("xT", [F, SHARD_PAD], bf16, kind="ExternalInput")
    wmat = nc.dram_tensor("wmat", [F, H], bf16, kind="ExternalInput")
    bias = nc.dram_tensor("bias", [H, 1], f32, kind="ExternalInput")
    iota = nc.dram_tensor("iota", [128, SUB], bf16, kind="ExternalInput")
    idx = nc.dram_tensor("idx", [128, cfg["e_pad"] // 16], i16, kind="ExternalInput")
    meta = nc.dram_tensor("meta", [128, cfg["nchunk"], 4], bf16, kind="ExternalInput")
    outT = nc.dram_tensor("outT", [H, OUTC], f32, kind="ExternalOutput")

    AF = mybir.ActivationFunctionType
    with tile.TileContext(nc) as tc:
        with (
            tc.tile_pool(name="const", bufs=1) as cpool,
            tc.tile_pool(name="xt", bufs=4) as xpool,
            tc.tile_pool(name="sup", bufs=2) as spool,
            tc.tile_pool(name="gath", bufs=10) as gpool,
            tc.tile_pool(name="ind", bufs=6) as ipool,
            tc.tile_pool(name="outb", bufs=2) as opool,
            tc.tile_pool(name="psum", bufs=8, space="PSUM") as ppool,
            tc.tile_pool(name="dram", bufs=1, space="DRAM") as dpool,
        ):
            nc.gpsimd.load_library(_mlp_lib)
            # 2 asymmetric AllGathers: quarter 0 -> small early table (run 0);
            # quarters 1-3 -> one big table holding runs 1-3. ~28us fixed cost
            # per collective makes more, finer AGs a net loss.
            shard0 = dpool.tile([TBLK, H], bf16, name="shard0")
            shard123 = dpool.tile([3 * TBLK, H], bf16, name="shard123")
            tableA = dpool.tile([TROWS, H], bf16, addr_space="Shared", name="tableA")
            tableB = dpool.tile(
                [3 * TROWS, H], bf16, addr_space="Shared", name="tableB"
            )

            w_sb = cpool.tile([128, KC, H], bf16)
            nc.sync.dma_start(
                out=w_sb[:], in_=wmat.ap().rearrange("(c k) h -> k c h", k=128)
            )
            bias_sb = cpool.tile([H, 1], f32)
            nc.sync.dma_start(out=bias_sb[:], in_=bias.ap())
            iota_sb = cpool.tile([128, SUB], bf16)
            nc.scalar.dma_start(out=iota_sb[:], in_=iota.ap())
            # preload the full idx / meta planes once (scalar HWDGE queue so
            # the sync engine starts the phase-1 x loads immediately)
            idx_sb = cpool.tile([128, cfg["e_pad"] // 16], i16)
            nc.scalar.dma_start(out=idx_sb[:], in_=idx.ap())
            meta_sb = cpool.tile([128, cfg["nchunk"], 4], bf16)
            nc.scalar.dma_start(out=meta_sb[:], in_=meta.ap())

            # ---- phase 1: support_shard = x_shard @ W; AG0 then AG1 ----
            for blk in range(N_RUNS):
                xts = []
                for kc in range(KC):
                    xt = xpool.tile([128, XBLK], bf16, tag=f"xt{kc}")
                    nc.sync.dma_start(
                        out=xt[:],
                        in_=xT.ap()[
                            kc * 128 : (kc + 1) * 128, blk * XBLK : (blk + 1) * XBLK
                        ],
                    )
                    xts.append(xt)
                st = spool.tile([128, XBLK], bf16)
                for i in range(XBLK // 128):
                    col = i * 128
                    ps = ppool.tile([128, 128], f32, tag="agg_ps")
                    for kc in range(KC):
                        nc.tensor.matmul(
                            ps[:],
                            xts[kc][:, col : col + 128],
                            w_sb[:, kc, :],
                            start=(kc == 0),
                            stop=(kc == KC - 1),
                        )
                    nc.scalar.activation(
                        out=st[:, col : col + 128], in_=ps[:], func=AF.Copy
                    )
                shard_dst = (
                    shard0[:]
                    if blk == 0
                    else shard123[(blk - 1) * TBLK : blk * TBLK, :]
                )
                nc.sync.dma_start(
                    out=shard_dst.rearrange("(i p) h -> p i h", p=128),
                    in_=st[:].rearrange("p (i h) -> p i h", h=H),
                )
                if blk == 0:
                    nc.gpsimd.collective_compute(
                        "AllGather",
                        mybir.AluOpType.bypass,
                        replica_groups=[list(range(N_CORES))],
                        ins=[shard0.opt()],
                        outs=[tableA.opt()],
                    )
                elif blk == N_RUNS - 1:
                    nc.gpsimd.collective_compute(
                        "AllGather",
                        mybir.AluOpType.bypass,
                        replica_groups=[list(range(N_CORES))],
                        ins=[shard123.opt()],
                        outs=[tableB.opt()],
                    )

            # ---- phase 2: gather + weighted-one-hot matmul segment sum ----
            # one psum tile per supertile; calls arrive in (group, t, S) order
            psums = {}
            mm_count = {}
            reg_cache = {}
            for ci, call in enumerate(cfg["calls"]):
                S, t, Gc = call["S"], call["t"], call["n_chunks"]
                L = Gc * 128
                q = ci % n_queues
                if S not in psums:
                    psums[S] = ppool.tile(
                        [128, PSUM_COLS], f32, name=f"agg_ps{S}", tag="agg_ps"
                    )
                    mm_count[S] = 0
                pss = psums[S]
                if L not in reg_cache:
                    reg_cache[L] = nc.gpsimd.to_reg(L)
                gt = gpool.tile([128, Gc, H], bf16)
                c0 = call["chunk_off"] * 8  # idx col = chunk_off*128/16
                src_ap = (
                    tableA[:]
                    if t == 0
                    else tableB[(t - 1) * TROWS : t * TROWS, :]
                )
                nc.gpsimd.dma_gather(
                    gt[:],
                    src_ap,
                    idx_sb[:, c0 : c0 + L // 16],
                    L,
                    reg_cache[L],
                    H,
                    single_packet=False,
                    queue_num=q,
                )
                mt = meta_sb[:, call["chunk_off"] : call["chunk_off"] + Gc, :]
                ind = ipool.tile([128, Gc, SUB], bf16)
                ind4 = ind[:].rearrange("p g (a b) -> p g a b", b=2)
                nc.vector.tensor_tensor(
                    out=ind4,
                    in0=iota_sb[:]
                    .rearrange("p (a b) -> p a b", b=2)[:, None, :, :]
                    .to_broadcast([128, Gc, SUB // 2, 2]),
                    in1=mt[:, :, None, 0:2].to_broadcast([128, Gc, SUB // 2, 2]),
                    op=mybir.AluOpType.is_equal,
                )
                nc.vector.tensor_tensor(
                    out=ind4,
                    in0=ind4,
                    in1=mt[:, :, None, 2:4].to_broadcast([128, Gc, SUB // 2, 2]),
                    op=mybir.AluOpType.mult,
                )
                total_S = 2 * N_RUNS * len(set(call["sub_local"]))
                for j in range(Gc):
                    so = call["sub_local"][j]
                    nc.tensor.matmul(
                        pss[:, so * SUB : (so + 1) * SUB],
                        gt[:, j, :],
                        ind[:, j, :],
                        start=(mm_count[S] == 0),
                        stop=(mm_count[S] == total_S - 1),
                    )
                    mm_count[S] += 1
                if mm_count[S] == total_S:
                    # supertile complete: epilogue
                    wS = min(PSUM_COLS, OUTC - S * PSUM_COLS)
                    ob = opool.tile([H, PSUM_COLS], f32)
                    nc.scalar.activation(
                        out=ob[:, :wS],
                        in_=pss[:, :wS],
                        func=AF.Relu,
                        bias=bias_sb[:],
                        scale=1.0,
                    )
                    nc.sync.dma_start(
                        out=outT.ap()[:, S * PSUM_COLS : S * PSUM_COLS + wS],
                        in_=ob[:, :wS],
                    )
                    del psums[S]
    nc.compile()
    return nc


def kernel(x, edge_src, edge_dst, edge_w, W, b):
    x = np.asarray(x)
    cfg, in_maps, col_of = prepare(x, edge_src, edge_dst, edge_w, W, b)
    nc = build_bass(cfg)
    want_trace = bool(int(os.environ.get("GNN_TRACE", "0")))
    core_ids = list(range(N_CORES))
    if want_trace:
        try:
            res = run_bass_kernel_spmd(nc, in_maps, core_ids=core_ids, trace=True)
        except Exception as e:
            print(f"traced run failed ({e}); retrying without trace")
            res = run_bass_kernel_spmd(nc, in_maps, core_ids=core_ids, trace=False)
    else:
        res = run_bass_kernel_spmd(nc, in_maps, core_ids=core_ids, trace=False)
    kernel.last_result = res
    out = np.empty((N_CORES * NPC, 128), np.float32)
    for c, r in enumerate(res.results):
        out[c * NPC : (c + 1) * NPC, :] = r["outT"].T[col_of[c], :]
    return np.ascontiguousarray(out)


kernel.last_result = None


# revision 12
# speedup vs baseline: 1.1045x; 1.0764x over previous
"""GNN message passing (GraphConvolution) on 8 TRN2 NeuronCores.

reference:
    support = x @ W                                   # [N, H]
    msgs    = support[edge_src] * edge_w[:, None]     # [E, H]
    agg     = segment_sum(msgs, edge_dst, N)          # [N, H]
    out     = relu(agg + b)

v3 strategy (dst-node 1D sharding; src-sharded support + AllGather):
  - Core c owns dst nodes [c*NPC, (c+1)*NPC).
  - Phase 1 is src-sharded: core c computes support rows for ITS x shard
    (12500 rows, padded 12800) in 4 quarter-blocks; after each block an
    AllGather builds table_k (25600 rows = 8 ranks x 3200), so phase-2
    gathers for run k start as soon as AG_k lands.
  - Host permutes each core's dst nodes into subtiles of <=64 nodes such
    that every (subtile, run) bucket has <=256 edges -> EXACTLY 2 chunks
    of 128 gather slots per bucket on every core (uniform SPMD shape,
    ~16% fewer gathered rows than the maxed-over-cores v2 layout).
  - Phase 2 is Q7-descriptor-generation bound (~2.4ns/row at 4-way SWDGE
    queue concurrency, measured): all per-call inputs (idx plane, meta
    plane) are preloaded ONCE, and num_idxs registers are hoisted, so the
    Pool engine issues gathers back-to-back with no interleaved waits
    (v2 lost 2x to issue-side serialization).
  - Weighted one-hot indicator S[e, d] = w_e * (dstloc_e == d), SUB=64
    wide, built on VectorE (2x_1P mode, never contends with GpSimd);
    each 128-slot chunk reduced with one TensorE matmul into a [128,512]
    PSUM supertile (8 subtiles); supers processed in groups of 6 ordered
    (run, super) to match the AG cadence with <=8 PSUM banks.
  - Epilogue: ScalarE relu(psum + b) -> outT [H, n_sub*64] -> host
    inverse-permutation.
"""

import os

import ml_dtypes
import numpy as np

import concourse.bass as bass
import concourse.mybir as mybir
import concourse.tile as tile
from concourse import bacc
from concourse.bass_utils import run_bass_kernel_spmd
from concourse.library_config import mlp as _mlp_lib

BF16 = ml_dtypes.bfloat16

N_CORES = 8
NPC = 12500  # dst nodes per core
SUB = 64  # dst nodes per subtile (one-hot width)
CAP = 256  # max edges per (subtile, run) bucket = 2 chunks
NODE_CAP = 64  # nodes per subtile
PSUM_COLS = 512  # one PSUM bank: 8 subtiles per supertile
SPS = PSUM_COLS // SUB  # subtiles per supertile (8)
GROUP = 6  # supertiles per scheduling group (psum bufs)

SHARD_PAD = 12800  # src rows per core, padded
N_RUNS = 4
TBLK = SHARD_PAD // N_RUNS  # 3200 shard rows per AG quarter
TROWS = TBLK * N_CORES  # 25600 rows per all-gathered table
XBLK = TBLK  # phase-1 row block == AG quarter


def _ceil_div(a, b):
    return (a + b - 1) // b


def _pack_core(dvec, n_sub):
    """Greedy 4-run balanced packing of this core's dst nodes into n_sub
    subtiles: per-bucket load <= CAP, per-subtile nodes <= NODE_CAP.
    dvec: [NPC, N_RUNS] per-node degree by run. Returns sub_of [NPC] or None."""
    tot = dvec.sum(axis=1)
    order = np.argsort(-tot, kind="stable")
    loads = np.zeros((n_sub, N_RUNS), np.int64)
    counts = np.zeros(n_sub, np.int64)
    sub_of = np.full(NPC, -1, np.int64)
    for n in order:
        d = dvec[n]
        cand = (counts < NODE_CAP) & (loads + d <= CAP).all(axis=1)
        if not cand.any():
            return None
        score = (loads + d).max(axis=1).astype(np.float64) + counts * 1e-4
        score[~cand] = np.inf
        b = int(np.argmin(score))
        loads[b] += d
        counts[b] += 1
        sub_of[n] = b
    return sub_of


def prepare(x, edge_src, edge_dst, edge_w, W, b):
    """Host-side sharding/routing. Returns (cfg, in_maps, col_of)."""
    n_nodes, nfeat = x.shape
    nhid = W.shape[1]
    assert n_nodes == N_CORES * NPC

    src = np.asarray(edge_src).astype(np.int64)
    dst = np.asarray(edge_dst).astype(np.int64)
    ew = np.asarray(edge_w).astype(np.float32)

    # src -> (run, row): run 0 = tableA (quarter 0, rank-major); runs 1-3 =
    # consecutive 25600-row slices of tableB (quarters 1-3, rank-major).
    s_rank = src // NPC
    s_loc = src % NPC
    s_q = s_loc // TBLK
    s_w = s_loc % TBLK
    rb = s_rank * (3 * TBLK) + (s_q - 1) * TBLK + s_w  # row in tableB (q>=1)
    s_run = np.where(s_q == 0, 0, 1 + rb // TROWS)
    s_row = np.where(s_q == 0, s_rank * TBLK + s_w, rb % TROWS)  # int16-safe

    core_of = dst // NPC

    # --- per-core balanced dst permutation ---
    n_sub = int(os.environ.get("GNN_NSUB", "204"))
    dvecs = []
    for c in range(N_CORES):
        m = core_of == c
        d_loc = dst[m] - c * NPC
        dvec = np.zeros((NPC, N_RUNS), np.int64)
        np.add.at(dvec, (d_loc, s_run[m]), 1)
        dvecs.append((m, d_loc, dvec))
    while True:
        percore = []
        for c in range(N_CORES):
            m, d_loc, dvec = dvecs[c]
            sub_of = _pack_core(dvec, n_sub)
            if sub_of is None:
                break
            percore.append((m, d_loc, sub_of))
        if len(percore) == N_CORES:
            break
        n_sub += 4

    n_super = _ceil_div(n_sub, SPS)
    # call program order: groups of supertiles, (run, super) within group
    call_order = []  # list of (S, t)
    for g0 in range(0, n_super, GROUP):
        gs = list(range(g0, min(g0 + GROUP, n_super)))
        for t in range(N_RUNS):
            for S in gs:
                call_order.append((S, t))
    # chunks per call: subtiles in super * 2 (except last, partial super)
    calls = []
    chunk_off = 0
    for S, t in call_order:
        subs = list(range(S * SPS, min(S * SPS + SPS, n_sub)))
        gc = 2 * len(subs)
        sub_local = []
        for s in subs:
            sub_local.extend([s - S * SPS, s - S * SPS])
        calls.append(
            dict(S=S, t=t, n_chunks=gc, chunk_off=chunk_off, sub_local=sub_local)
        )
        chunk_off += gc
    nchunk = chunk_off
    e_pad = nchunk * 128
    assert nchunk == n_sub * N_RUNS * 2

    # chunk slot ranges per (subtile, run): chunk index within program order
    chunk_of = {}
    pos = 0
    for call in calls:
        S, t = call["S"], call["t"]
        subs = range(S * SPS, min(S * SPS + SPS, n_sub))
        for s in subs:
            chunk_of[(s, t)] = pos
            pos += 2
    assert pos == nchunk

    in_maps = []
    w_bf = np.ascontiguousarray(np.asarray(W, np.float32).astype(BF16))
    bias = np.asarray(b, np.float32).reshape(nhid, 1).copy()
    iota = np.tile(np.arange(SUB, dtype=np.float32).astype(BF16)[None, :], (128, 1))
    x_np = np.asarray(x, np.float32)
    col_of = np.zeros((N_CORES, NPC), np.int64)
    for c in range(N_CORES):
        m, d_loc, sub_of = percore[c]
        # within-subtile slot assignment (stable by node id)
        slot = np.zeros(NPC, np.int64)
        for s in range(n_sub):
            nodes = np.nonzero(sub_of == s)[0]
            slot[nodes] = np.arange(len(nodes))
        col_of[c] = sub_of * SUB + slot

        e_sub = sub_of[d_loc]  # subtile of each edge
        e_dl = slot[d_loc]  # within-subtile one-hot position
        e_t = s_run[m]
        e_row = s_row[m]
        e_w = ew[m]
        key = e_sub * N_RUNS + e_t
        order = np.argsort(key, kind="stable")
        e_sub, e_dl, e_t, e_row, e_w = (
            a[order] for a in (e_sub, e_dl, e_t, e_row, e_w)
        )
        cnt = np.bincount(key[order], minlength=n_sub * N_RUNS)
        seg_start = np.zeros(n_sub * N_RUNS + 1, np.int64)
        np.cumsum(cnt, out=seg_start[1:])

        idx_pad = np.zeros(e_pad, np.int16)
        dl_pad = np.zeros(e_pad, np.float32)
        ww_pad = np.zeros(e_pad, np.float32)
        for s in range(n_sub):
            for t in range(N_RUNS):
                k = s * N_RUNS + t
                a0, a1 = seg_start[k], seg_start[k + 1]
                n = a1 - a0
                if n == 0:
                    continue
                off = chunk_of[(s, t)] * 128
                idx_pad[off : off + n] = e_row[a0:a1].astype(np.int16)
                dl_pad[off : off + n] = e_dl[a0:a1].astype(np.float32)
                ww_pad[off : off + n] = e_w[a0:a1]

        xT_shard = np.zeros((nfeat, SHARD_PAD), BF16)
        xT_shard[:, :NPC] = x_np[c * NPC : (c + 1) * NPC, :].T.astype(BF16)
        in_maps.append(
            dict(
                xT=xT_shard,
                wmat=w_bf,
                bias=bias,
                iota=iota,
                idx=np.ascontiguousarray(np.tile(idx_pad.reshape(-1, 16).T, (8, 1))),
                # dl/ew duplicated in adjacent pairs (DVE 2x_1P needs step-1
                # 2-elem innermost runs), merged into one plane
                meta=np.ascontiguousarray(
                    np.concatenate(
                        [
                            np.repeat(
                                a.reshape(-1, 128).T.astype(BF16)[:, :, None], 2, 2
                            )
                            for a in (dl_pad, ww_pad)
                        ],
                        axis=2,
                    )
                ),
            )
        )

    cfg = dict(
        nfeat=nfeat,
        nhid=nhid,
        n_sub=n_sub,
        n_super=n_super,
        nchunk=nchunk,
        e_pad=e_pad,
        calls=calls,
    )
    return cfg, in_maps, col_of


def build_bass(cfg):
    F, H = cfg["nfeat"], cfg["nhid"]
    KC = F // 128
    n_sub = cfg["n_sub"]
    OUTC = n_sub * SUB
    assert F % 128 == 0 and H == 128
    n_queues = int(os.environ.get("GNN_QUEUES", "4"))

    nc = bacc.Bacc(
        "TRN2",
        target_bir_lowering=False,
        debug=False,
        enable_asserts=True,
        num_devices=N_CORES,
        num_swdge_queues=n_queues,
    )
    f32, bf16, i16 = mybir.dt.float32, mybir.dt.bfloat16, mybir.dt.int16
    xT = nc.dram_tensor("xT", [F, SHARD_PAD], bf16, kind="ExternalInput")
    wmat = nc.dram_tensor("wmat", [F, H], bf16, kind="ExternalInput")
    bias = nc.dram_tensor("bias", [H, 1], f32, kind="ExternalInput")
    iota = nc.dram_tensor("iota", [128, SUB], bf16, kind="ExternalInput")
    idx = nc.dram_tensor("idx", [128, cfg["e_pad"] // 16], i16, kind="ExternalInput")
    meta = nc.dram_tensor("meta", [128, cfg["nchunk"], 4], bf16, kind="ExternalInput")
    outT = nc.dram_tensor("outT", [H, OUTC], f32, kind="ExternalOutput")

    AF = mybir.ActivationFunctionType
    with tile.TileContext(nc) as tc:
        with (
            tc.tile_pool(name="const", bufs=1) as cpool,
            tc.tile_pool(name="xt", bufs=4) as xpool,
            tc.tile_pool(name="sup", bufs=2) as spool,
            tc.tile_pool(name="gath", bufs=14) as gpool,
            tc.tile_pool(name="ind", bufs=10) as ipool,
            tc.tile_pool(name="outb", bufs=2) as opool,
            tc.tile_pool(name="psum", bufs=8, space="PSUM") as ppool,
            tc.tile_pool(name="dram", bufs=1, space="DRAM") as dpool,
        ):
            nc.gpsimd.load_library(_mlp_lib)
            # 2 asymmetric AllGathers: quarter 0 -> small early table (run 0);
            # quarters 1-3 -> one big table holding runs 1-3. ~28us fixed cost
            # per collective makes more, finer AGs a net loss.
            shard0 = dpool.tile([TBLK, H], bf16, name="shard0")
            shard123 = dpool.tile([3 * TBLK, H], bf16, name="shard123")
            tableA = dpool.tile([TROWS, H], bf16, addr_space="Shared", name="tableA")
            tableB = dpool.tile(
                [3 * TROWS, H], bf16, addr_space="Shared", name="tableB"
            )

            w_sb = cpool.tile([128, KC, H], bf16)
            nc.sync.dma_start(
                out=w_sb[:], in_=wmat.ap().rearrange("(c k) h -> k c h", k=128)
            )
            bias_sb = cpool.tile([H, 1], f32)
            nc.sync.dma_start(out=bias_sb[:], in_=bias.ap())
            iota_sb = cpool.tile([128, SUB], bf16)
            nc.scalar.dma_start(out=iota_sb[:], in_=iota.ap())
            # preload the full idx / meta planes once (scalar HWDGE queue so
            # the sync engine starts the phase-1 x loads immediately)
            idx_sb = cpool.tile([128, cfg["e_pad"] // 16], i16)
            nc.scalar.dma_start(out=idx_sb[:], in_=idx.ap())
            meta_sb = cpool.tile([128, cfg["nchunk"], 4], bf16)
            nc.scalar.dma_start(out=meta_sb[:], in_=meta.ap())

            # ---- phase 1: support_shard = x_shard @ W; AG0 then AG1 ----
            for blk in range(N_RUNS):
                xts = []
                for kc in range(KC):
                    xt = xpool.tile([128, XBLK], bf16, tag=f"xt{kc}")
                    nc.sync.dma_start(
                        out=xt[:],
                        in_=xT.ap()[
                            kc * 128 : (kc + 1) * 128, blk * XBLK : (blk + 1) * XBLK
                        ],
                    )
                    xts.append(xt)
                st = spool.tile([128, XBLK], bf16)
                for i in range(XBLK // 128):
                    col = i * 128
                    ps = ppool.tile([128, 128], f32, tag="agg_ps")
                    for kc in range(KC):
                        nc.tensor.matmul(
                            ps[:],
                            xts[kc][:, col : col + 128],
                            w_sb[:, kc, :],
                            start=(kc == 0),
                            stop=(kc == KC - 1),
                        )
                    nc.scalar.activation(
                        out=st[:, col : col + 128], in_=ps[:], func=AF.Copy
                    )
                shard_dst = (
                    shard0[:]
                    if blk == 0
                    else shard123[(blk - 1) * TBLK : blk * TBLK, :]
                )
                nc.sync.dma_start(
                    out=shard_dst.rearrange("(i p) h -> p i h", p=128),
                    in_=st[:].rearrange("p (i h) -> p i h", h=H),
                )
                if blk == 0:
                    nc.gpsimd.collective_compute(
                        "AllGather",
                        mybir.AluOpType.bypass,
                        replica_groups=[list(range(N_CORES))],
                        ins=[shard0.opt()],
                        outs=[tableA.opt()],
                    )
                elif blk == N_RUNS - 1:
                    nc.gpsimd.collective_compute(
                        "AllGather",
                        mybir.AluOpType.bypass,
                        replica_groups=[list(range(N_CORES))],
                        ins=[shard123.opt()],
                        outs=[tableB.opt()],
                    )

            # ---- phase 2: gather + weighted-one-hot matmul segment sum ----
            # one psum tile per supertile; calls arrive in (group, t, S) order
            psums = {}
            mm_count = {}
            reg_cache = {}
            for ci, call in enumerate(cfg["calls"]):
                S, t, Gc = call["S"], call["t"], call["n_chunks"]
                L = Gc * 128
                q = ci % n_queues
                if S not in psums:
                    psums[S] = ppool.tile(
                        [128, PSUM_COLS], f32, name=f"agg_ps{S}", tag="agg_ps"
                    )
                    mm_count[S] = 0
                pss = psums[S]
                if L not in reg_cache:
                    reg_cache[L] = nc.gpsimd.to_reg(L)
                gt = gpool.tile([128, Gc, H], bf16)
                c0 = call["chunk_off"] * 8  # idx col = chunk_off*128/16
                src_ap = (
                    tableA[:]
                    if t == 0
                    else tableB[(t - 1) * TROWS : t * TROWS, :]
                )
                nc.gpsimd.dma_gather(
                    gt[:],
                    src_ap,
                    idx_sb[:, c0 : c0 + L // 16],
                    L,
                    reg_cache[L],
                    H,
                    single_packet=False,
                    queue_num=q,
                )
                mt = meta_sb[:, call["chunk_off"] : call["chunk_off"] + Gc, :]
                ind = ipool.tile([128, Gc, SUB], bf16)
                ind4 = ind[:].rearrange("p g (a b) -> p g a b", b=2)
                nc.vector.tensor_tensor(
                    out=ind4,
                    in0=iota_sb[:]
                    .rearrange("p (a b) -> p a b", b=2)[:, None, :, :]
                    .to_broadcast([128, Gc, SUB // 2, 2]),
                    in1=mt[:, :, None, 0:2].to_broadcast([128, Gc, SUB // 2, 2]),
                    op=mybir.AluOpType.is_equal,
                )
                nc.vector.tensor_tensor(
                    out=ind4,
                    in0=ind4,
                    in1=mt[:, :, None, 2:4].to_broadcast([128, Gc, SUB // 2, 2]),
                    op=mybir.AluOpType.mult,
                )
                total_S = 2 * N_RUNS * len(set(call["sub_local"]))
                for j in range(Gc):
                    so = call["sub_local"][j]
                    nc.tensor.matmul(
                        pss[:, so * SUB : (so + 1) * SUB],
                        gt[:, j, :],
                        ind[:, j, :],
                        start=(mm_count[S] == 0),
                        stop=(mm_count[S] == total_S - 1),
                    )
                    mm_count[S] += 1
                if mm_count[S] == total_S:
                    # supertile complete: epilogue
                    wS = min(PSUM_COLS, OUTC - S * PSUM_COLS)
                    ob = opool.tile([H, PSUM_COLS], f32)
                    nc.scalar.activation(
                        out=ob[:, :wS],
                        in_=pss[:, :wS],
                        func=AF.Relu,
                        bias=bias_sb[:],
                        scale=1.0,
                    )
                    nc.sync.dma_start(
                        out=outT.ap()[:, S * PSUM_COLS : S * PSUM_COLS + wS],
                        in_=ob[:, :wS],
                    )
                    del psums[S]
    nc.compile()
    return nc


def kernel(x, edge_src, edge_dst, edge_w, W, b):
    x = np.asarray(x)
    cfg, in_maps, col_of = prepare(x, edge_src, edge_dst, edge_w, W, b)
    nc = build_bass(cfg)
    want_trace = bool(int(os.environ.get("GNN_TRACE", "0")))
    core_ids = list(range(N_CORES))
    if want_trace:
        try:
            res = run_bass_kernel_spmd(nc, in_maps, core_ids=core_ids, trace=True)
        except Exception as e:
            print(f"traced run failed ({e}); retrying without trace")
            res = run_bass_kernel_spmd(nc, in_maps, core_ids=core_ids, trace=False)
    else:
        res = run_bass_kernel_spmd(nc, in_maps, core_ids=core_ids, trace=False)
    kernel.last_result = res
    out = np.empty((N_CORES * NPC, 128), np.float32)
    for c, r in enumerate(res.results):
        out[c * NPC : (c + 1) * NPC, :] = r["outT"].T[col_of[c], :]
    return np.ascontiguousarray(out)


kernel.last_result = None
